# revision 1
# baseline (speedup 1.0000x reference)
"""Trainium2 Bass kernel for BaseCausalWanSelfAttention (local+sink sparse attention
with interleaved rotary), SPMD across 8 NeuronCores.

Sharding: the 24 (batch, head) pairs are split 3-per-core across 8 cores; each
core runs full local+sink attention for its pairs independently (no collectives).
"""
import sys

sys.path.insert(0, "/opt/trn_rl_repo")

import numpy as np

import concourse.bacc as bacc
import concourse.tile as tile
import concourse.mybir as mybir

dt = mybir.dt

# Problem config (hardcoded per contest contract)
B, S, H, D = 2, 3072, 12, 128
LOCAL_WINDOW = 1560
SINK = 128
N_CORES = 8
PER_CORE = (B * H) // N_CORES  # 3
QB = 512  # query block (columns of transposed scores)
NQC = QB // 128  # 128-query chunks per block
SCALE = 1.0 / float(np.sqrt(D))


def _window_partial_deltas(w):
    """k-tile offsets (qi - kj) where the local-window edge cuts through the
    128x128 tile; maps delta -> threshold T with allowed iff (c - p) < T."""
    out = {}
    for d in range((w - 127 + 127) // 128, (w + 127) // 128 + 1):
        t = w - 128 * d
        if -127 <= t <= 127:
            out[d] = t
    return out


def chunk_kinds(qb, kj, w=LOCAL_WINDOW, nqc=NQC):
    """Per 128-query chunk classification of k-tile kj for query block qb.
    Returns list of (t, kind) with kind in {"full", "diag", ("win", delta)} for
    valid chunks only. SINK==128 assumed (k-tile 0 fully attendable)."""
    partial = _window_partial_deltas(w)
    max_delta = max(partial) if partial else (w - 1) // 128
    kinds = []
    for t in range(nqc):
        qi = nqc * qb + t
        if kj == 0:
            kinds.append((t, "diag" if qi == 0 else "full"))
            continue
        delta = qi - kj
        if delta < 0 or delta > max_delta:
            continue
        if delta == 0:
            kinds.append((t, "diag"))
        elif delta in partial:
            kinds.append((t, ("win", delta)))
        else:
            kinds.append((t, "full"))
    return kinds


def kj_list(qb, s=S, w=LOCAL_WINDOW, nqc=NQC):
    partial = _window_partial_deltas(w)
    max_delta = max(partial) if partial else (w - 1) // 128
    n_ktiles = s // 128
    hi = min(nqc * qb + nqc - 1, n_ktiles - 1)
    lo = max(1, nqc * qb - max_delta)
    return [0] + [kj for kj in range(lo, hi + 1)]


def build_nc(s=S, per_core=PER_CORE, w=LOCAL_WINDOW):
    """Build the SPMD single-core program (identical on all cores)."""
    nqb = s // QB
    partial = _window_partial_deltas(w)

    nc = bacc.Bacc("TRN2", target_bir_lowering=False, debug=False)

    qT = nc.declare_dram_parameter("qT", [per_core, 128, s], dt.float32r, isOutput=False)
    kT = nc.declare_dram_parameter("kT", [per_core, 128, s], dt.float32r, isOutput=False)
    v = nc.declare_dram_parameter("v", [per_core, s, 128], dt.float32r, isOutput=False)
    cexpT = nc.declare_dram_parameter("cexpT", [128, s], dt.float32r, isOutput=False)
    ssigT = nc.declare_dram_parameter("ssigT", [128, s], dt.float32r, isOutput=False)
    pswap = nc.declare_dram_parameter("pswap", [128, 128], dt.float32r, isOutput=False)
    ident = nc.declare_dram_parameter("ident", [128, 128], dt.float32, isOutput=False)
    ones = nc.declare_dram_parameter("ones", [128, 128], dt.float32r, isOutput=False)
    maskD = nc.declare_dram_parameter("maskD", [128, 128], dt.float32r, isOutput=False)
    wmask_names = {}
    for delta in sorted(partial):
        nm = f"maskW{delta}"
        wmask_names[delta] = nc.declare_dram_parameter(
            nm, [128, 128], dt.float32r, isOutput=False
        )
    out = nc.declare_dram_parameter("out", [per_core, s, 128], dt.float32, isOutput=True)

    with tile.TileContext(nc) as tc:
        with (
            tc.tile_pool(name="const", bufs=1) as cpool,
            tc.tile_pool(name="big", bufs=2) as bigpool,
            tc.tile_pool(name="probs", bufs=7) as ppool,
            tc.tile_pool(name="tail", bufs=2) as tpool,
            tc.tile_pool(name="ps_sc", bufs=5, space="PSUM") as ps_sc,
            tc.tile_pool(name="ps_out", bufs=2, space="PSUM") as ps_out,
            tc.tile_pool(name="ps_den", bufs=1, space="PSUM") as ps_den,
        ):
            # constants
            cexp_sb = cpool.tile([128, s], dt.float32r, tag="cexp")
            ssig_sb = cpool.tile([128, s], dt.float32r, tag="ssig")
            pswap_sb = cpool.tile([128, 128], dt.float32r, tag="pswap")
            nc.sync.dma_start(out=pswap_sb[:], in_=pswap[:])
            nc.sync.dma_start(out=cexp_sb[:, 0:1024], in_=cexpT[:, 0:1024])
            nc.sync.dma_start(out=ssig_sb[:, 0:1024], in_=ssigT[:, 0:1024])
            ident_sb = cpool.tile([128, 128], dt.float32, tag="ident")
            ones_sb = cpool.tile([128, 128], dt.float32r, tag="ones")
            maskD_sb = cpool.tile([128, 128], dt.float32r, tag="maskD")
            wdeltas = sorted(wmask_names)
            wmask_sb = {
                delta: cpool.tile(
                    [128, 128], dt.float32r, tag=f"maskW{delta}", name=f"mW{delta}"
                )
                for delta in wdeltas
            }
            wpair_sb = None
            if len(wdeltas) == 2 and wdeltas[1] == wdeltas[0] + 1:
                wpair_sb = cpool.tile([128, 256], dt.float32r, tag="maskWpair")

            def load_consts_rest():
                nc.sync.dma_start(out=ident_sb[:], in_=ident[:])
                nc.sync.dma_start(out=ones_sb[:], in_=ones[:])
                nc.sync.dma_start(out=maskD_sb[:], in_=maskD[:])
                for delta, m in wmask_sb.items():
                    nc.sync.dma_start(out=m[:], in_=wmask_names[delta][:])
                if wpair_sb is not None:
                    nc.sync.dma_start(
                        out=wpair_sb[:, 0:128], in_=wmask_names[wdeltas[0]][:]
                    )
                    nc.sync.dma_start(
                        out=wpair_sb[:, 128:256], in_=wmask_names[wdeltas[1]][:]
                    )
                for c2 in range(1, s // 1024):
                    sl2 = slice(c2 * 1024, (c2 + 1) * 1024)
                    nc.sync.dma_start(out=cexp_sb[:, sl2], in_=cexpT[:, sl2])
                    nc.sync.dma_start(out=ssig_sb[:, sl2], in_=ssigT[:, sl2])

            def load(u):
                qraw = bigpool.tile([128, s], dt.float32r, tag="qraw")
                kraw = bigpool.tile([128, s], dt.float32r, tag="kraw")
                v_sb = bigpool.tile([128, s], dt.float32r, tag="v")
                for c2 in range(s // 1024):
                    sl2 = slice(c2 * 1024, (c2 + 1) * 1024)
                    nc.sync.dma_start(out=qraw[:, sl2], in_=qT[u][:, sl2])
                    nc.sync.dma_start(out=kraw[:, sl2], in_=kT[u][:, sl2])
                nc.sync.dma_start(
                    out=v_sb[:].rearrange("p (n d) -> p n d", d=128),
                    in_=v[u].rearrange("(n p) d -> p n d", p=128),
                )
                rq = bigpool.tile([128, s], dt.float32r, tag="rq")
                rk = bigpool.tile([128, s], dt.float32r, tag="rk")
                return qraw, kraw, v_sb, rq, rk

            def rotary(tiles, lo, hi):
                """Rotary for columns [lo,hi) of q and k; 512-col DVE chunks."""
                qraw, kraw, v_sb, rq, rk = tiles
                for raw, r in ((qraw, rq), (kraw, rk)):
                    step = 1024 if (hi - lo) % 1024 == 0 else 512
                    for c in range(lo // step, hi // step):
                        sl = slice(c * step, (c + 1) * step)
                        sws = []
                        for h2 in range(step // 512):
                            ssl = slice(c * step + h2 * 512, c * step + (h2 + 1) * 512)
                            sw = ps_sc.tile([128, 512], dt.float32, tag="sc")
                            nc.tensor.matmul(
                                sw[:], pswap_sb[:], raw[:, ssl], start=True, stop=True
                            )
                            sws.append((ssl, sw))
                        # r = raw * cexp
                        nc.vector.tensor_mul(r[:, sl], raw[:, sl], cexp_sb[:, sl])
                        # raw <- swap(raw) * ssig  (psum src; raw reused as scratch)
                        for ssl, sw in sws:
                            nc.vector.tensor_mul(
                                raw[:, ssl], sw[:].bitcast(dt.float32r), ssig_sb[:, ssl]
                            )
                        # r += scratch
                        nc.vector.tensor_add(r[:, sl], r[:, sl], raw[:, sl])

            def emit_masks(probs, kinds):
                mk = [k for k in kinds if k[1] != "full"]
                j = 0
                while j < len(mk):
                    t, kind = mk[j]
                    if (
                        wpair_sb is not None
                        and j + 1 < len(mk)
                        and kind != "diag"
                        and mk[j + 1][1] != "diag"
                        and mk[j + 1][0] == t + 1
                        and kind[1] == wdeltas[0]
                    ):
                        tsl = slice(t * 128, (t + 2) * 128)
                        nc.vector.tensor_mul(probs[:, tsl], probs[:, tsl], wpair_sb[:])
                        j += 2
                        continue
                    m = maskD_sb if kind == "diag" else wmask_sb[kind[1]]
                    tsl = slice(t * 128, (t + 1) * 128)
                    nc.vector.tensor_mul(probs[:, tsl], probs[:, tsl], m[:])
                    j += 1

            def qb_order(qb):
                kjs = kj_list(qb, s=s, w=w)
                tiles = []
                for kj in kjs:
                    kinds = chunk_kinds(qb, kj, w=w)
                    assert kinds, (qb, kj)
                    tiles.append((kj, kinds, kinds[0][0], kinds[-1][0] + 1))
                fulls = [x for x in tiles if x[3] - x[2] == NQC]
                parts = [x for x in tiles if x[3] - x[2] != NQC]
                assert fulls[0][0] == 0
                order = [fulls[0]]
                rest_f = fulls[1:]
                rest_p = list(parts)
                stride = (
                    max(1, len(rest_f) // (len(rest_p) + 1))
                    if rest_p
                    else len(rest_f) or 1
                )
                while rest_f or rest_p:
                    order.extend(rest_f[:stride])
                    rest_f = rest_f[stride:]
                    if rest_p:
                        order.append(rest_p.pop(0))
                return order

            WAVE = 3
            state = {"pv": [], "tail": None}

            def flush_pv():
                if state["pv"]:
                    state["pv"].pop(0)()

            def flush_all():
                while state["pv"]:
                    flush_pv()

            def attention_qb(u, rq, rk, v_sb, qb):
                order = qb_order(qb)
                n_tiles = len(order)
                qbctx = {}

                def get_psums():
                    if "outT" not in qbctx:
                        outT_ps = ps_out.tile([128, QB], dt.float32, tag="outT")
                        den_ps = ps_den.tile([128, QB], dt.float32, tag="den")
                        qbctx["outT"] = outT_ps
                        qbctx["den"] = den_ps
                    return qbctx["outT"], qbctx["den"]

                for w0 in range(0, n_tiles, WAVE):
                    wave = order[w0 : w0 + WAVE]
                    wprobs = []
                    for kj, kinds, t0, t1 in wave:
                        csl = slice(qb * QB + t0 * 128, qb * QB + t1 * 128)
                        psl = slice(t0 * 128, t1 * 128)
                        ksl = slice(kj * 128, (kj + 1) * 128)
                        sc = ps_sc.tile([128, QB], dt.float32, tag="sc")
                        nc.tensor.matmul(
                            sc[:, psl], rk[:, ksl], rq[:, csl], start=True, stop=True
                        )
                        probs = ppool.tile([128, QB], dt.float32r, tag="probs")
                        nc.scalar.activation(
                            probs[:, psl],
                            sc[:, psl],
                            mybir.ActivationFunctionType.Exp,
                            scale=SCALE,
                        )
                        emit_masks(probs, kinds)
                        wprobs.append(probs)

                    is_last_wave = w0 + WAVE >= n_tiles

                    def pv_emit(
                        u=u, qb=qb, wave=wave, wprobs=wprobs,
                        w0=w0, n_tiles=n_tiles, last_wave=is_last_wave,
                    ):
                        outT_ps, den_ps = get_psums()
                        for wi, (kj, kinds, t0, t1) in enumerate(wave):
                            psl = slice(t0 * 128, t1 * 128)
                            ksl = slice(kj * 128, (kj + 1) * 128)
                            first = kj == 0
                            last = w0 + wi == n_tiles - 1
                            nc.tensor.matmul(
                                outT_ps[:, psl], v_sb[:, ksl], wprobs[wi][:, psl],
                                start=first, stop=last,
                            )
                            nc.tensor.matmul(
                                den_ps[:, psl], ones_sb[:], wprobs[wi][:, psl],
                                start=first, stop=last,
                            )
                        if last_wave:
                            # normalize now; transposes/store deferred one qb
                            rden = tpool.tile([128, QB], dt.float32, tag="rden")
                            nc.vector.reciprocal_approx_fast(rden[:], den_ps[:])
                            outN = tpool.tile([128, QB], dt.float32, tag="outN")
                            nc.vector.tensor_mul(outN[:], outT_ps[:], rden[:])

                            def tail(u=u, qb=qb, outN=outN):
                                tr = ps_sc.tile([128, QB], dt.float32, tag="sc")
                                for c in range(NQC):
                                    tsl = slice(c * 128, (c + 1) * 128)
                                    nc.tensor.transpose(
                                        tr[:, tsl], outN[:, tsl], ident_sb[:]
                                    )
                                out_sb = tpool.tile([128, QB], dt.float32, tag="out_sb")
                                nc.scalar.copy(out_sb[:], tr[:])
                                nc.sync.dma_start(
                                    out=out[u].rearrange("(n p) d -> p n d", p=128)[
                                        :, qb * NQC : (qb + 1) * NQC, :
                                    ],
                                    in_=out_sb[:].rearrange("p (n d) -> p n d", d=128),
                                )

                            if state["tail"] is not None:
                                state["tail"]()
                            state["tail"] = tail

                    state["pv"].append(pv_emit)
                    flush_pv() if len(state["pv"]) > 1 else None

            cur = load(0)
            load_consts_rest()
            for u in range(per_core):
                nxt = load(u + 1) if u + 1 < per_core else None
                for qb in range(nqb):
                    if u == 0:
                        rotary(cur, qb * QB, (qb + 1) * QB)
                    attention_qb(u, cur[3], cur[4], cur[2], qb)
                if nxt is not None:
                    rotary(nxt, 0, s)
                cur = nxt
            flush_all()
            if state["tail"] is not None:
                state["tail"]()

    nc.compile()
    return nc


def host_prep(q, k, v, cos, sin, s=S, w=LOCAL_WINDOW):
    """Build per-core input maps from full inputs."""
    b, _, h, d = q.shape
    partial = _window_partial_deltas(w)

    cexp = np.empty((128, s), dtype=np.float32)
    ssig = np.empty((128, s), dtype=np.float32)
    cexp[0::2, :] = cos.T
    cexp[1::2, :] = cos.T
    ssig[0::2, :] = -sin.T
    ssig[1::2, :] = sin.T

    pswap = np.zeros((128, 128), dtype=np.float32)
    idx = np.arange(128)
    pswap[idx, idx ^ 1] = 1.0
    ident = np.eye(128, dtype=np.float32)
    ones = np.ones((128, 128), dtype=np.float32)

    p = np.arange(128)[:, None]
    c = np.arange(128)[None, :]
    maskD = (c >= p).astype(np.float32)
    wmasks = {
        delta: ((c - p) < t).astype(np.float32) for delta, t in partial.items()
    }

    units = [(bi, hi) for bi in range(b) for hi in range(h)]
    per = len(units) // N_CORES
    in_maps = []
    for core in range(N_CORES):
        us = units[core * per : (core + 1) * per]
        qTc = np.ascontiguousarray(
            np.stack([q[bi, :, hi, :].T for bi, hi in us])
        )
        kTc = np.ascontiguousarray(
            np.stack([k[bi, :, hi, :].T for bi, hi in us])
        )
        vc = np.ascontiguousarray(np.stack([v[bi, :, hi, :] for bi, hi in us]))
        m = {
            "qT": qTc,
            "kT": kTc,
            "v": vc,
            "cexpT": cexp,
            "ssigT": ssig,
            "pswap": pswap,
            "ident": ident,
            "ones": ones,
            "maskD": maskD,
        }
        for delta, msk in wmasks.items():
            m[f"maskW{delta}"] = msk
        in_maps.append(m)
    return in_maps, units


_NC_CACHE = {}


def kernel(q, k, v, cos, sin):
    from concourse.bass_utils import run_bass_kernel_spmd

    q = np.asarray(q, dtype=np.float32)
    k = np.asarray(k, dtype=np.float32)
    v = np.asarray(v, dtype=np.float32)
    cos = np.asarray(cos, dtype=np.float32)
    sin = np.asarray(sin, dtype=np.float32)

    if "nc" not in _NC_CACHE:
        _NC_CACHE["nc"] = build_nc()
    nc = _NC_CACHE["nc"]

    in_maps, units = host_prep(q, k, v, cos, sin)
    res = run_bass_kernel_spmd(nc, in_maps, core_ids=list(range(N_CORES)))

    b, s, h, d = q.shape
    full = np.empty((b, s, h, d), dtype=np.float32)
    per = len(units) // N_CORES
    for core in range(N_CORES):
        o = res.results[core]["out"]  # [per, s, 128]
        for i, (bi, hi) in enumerate(units[core * per : (core + 1) * per]):
            full[bi, :, hi, :] = o[i]
    return full



# revision 8
# speedup vs baseline: 1.4979x; 1.4979x over previous
"""Trainium2 Bass kernel for BaseCausalWanSelfAttention (local+sink sparse attention
with interleaved rotary), SPMD across 8 NeuronCores.

Sharding: the 24 (batch, head) pairs are split 3-per-core across 8 cores; each
core runs full local+sink attention for its pairs independently (no collectives).

v3: host-side rotary + layouts; fp16 datapath on device; denominator via
probs-accumulation (S) + one ones-matmul per query block; wide exp groups;
narrowed delta-13 window tiles.
"""
import sys

sys.path.insert(0, "/opt/trn_rl_repo")

import numpy as np

import concourse.bacc as bacc
import concourse.tile as tile
import concourse.mybir as mybir

dt = mybir.dt

# Problem config (hardcoded per contest contract)
B, S, H, D = 2, 3072, 12, 128
LOCAL_WINDOW = 1560
SINK = 128
N_CORES = 8
PER_CORE = (B * H) // N_CORES  # 3
QB = 512  # query block
NQC = QB // 128  # 128-query chunks per block
NKT = S // 128  # 24 key tiles
SCALE = 1.0 / float(np.sqrt(D))

# window partial deltas for w=1560: {12: 24, 13: -104}
DELTA_W12 = 12
T_W12 = LOCAL_WINDOW - 128 * DELTA_W12  # 24
DELTA_W13 = 13
T_W13 = LOCAL_WINDOW - 128 * DELTA_W13  # -104
W13_W = 128 + T_W13  # 24: only query cols [0,24) of a win13 chunk are live
MAX_DELTA = DELTA_W13

GROUP_W = 1024  # exp group width (2 PSUM banks)
# accum adds with eff width <= this go to gpsimd (Pool engine) instead of DVE
GSIMD_ACCUM_MAX_W = 0  # disabled initially; tune later


def chunk_kind(qi, kj):
    """Kind of the 128x128 block (key tile kj, query chunk qi):
    None | "full" | "diag" | "w12" | "w13"."""
    if kj == 0:
        return "diag" if qi == 0 else "full"
    delta = qi - kj
    if delta < 0 or delta > MAX_DELTA:
        return None
    if delta == 0:
        return "diag"
    if delta == DELTA_W12:
        return "w12"
    if delta == DELTA_W13:
        return "w13"
    return "full"


def qb_tiles(qb):
    """Tiles for query block qb: list of dicts with kj, t0, t1 (chunk span),
    kinds (per chunk), eff_w (narrowed width)."""
    lo = max(1, NQC * qb - MAX_DELTA)
    hi = min(NQC * qb + NQC - 1, NKT - 1)
    out = []
    for kj in [0] + list(range(lo, hi + 1)):
        kinds = []
        for t in range(NQC):
            k = chunk_kind(NQC * qb + t, kj)
            if k is not None:
                kinds.append((t, k))
        if not kinds:
            continue
        t0 = kinds[0][0]
        t1 = kinds[-1][0] + 1
        assert len(kinds) == t1 - t0, (qb, kj, kinds)
        eff_w = 128 * (t1 - t0)
        if kinds[-1][1] == "w13":
            eff_w -= 128 - W13_W
        out.append(dict(kj=kj, t0=t0, t1=t1, kinds=kinds, eff_w=eff_w))
    return out


def plan_groups(tiles):
    """Pack tiles into exp groups of width <= GROUP_W; each tile's QK matmul
    must not cross a 512-col PSUM bank boundary. Returns list of groups; each
    group is (spans, [(tile, off)]) where spans are the contiguous written
    column ranges (exp is emitted per span — pads are never written/read)."""
    groups = []
    cur = []
    spans = []
    span_start = 0
    pos = 0

    def close_group():
        nonlocal cur, spans, span_start, pos
        if pos > span_start:
            spans.append((span_start, pos))
        if cur:
            groups.append((spans, cur))
        cur, spans, span_start, pos = [], [], 0, 0

    for tl in tiles:
        w = tl["eff_w"]
        assert w <= 512
        start = pos
        if (start % 512) + w > 512:  # would cross a bank boundary
            start = ((start // 512) + 1) * 512
        if start + w > GROUP_W:
            close_group()
            start = 0
        if start != pos:  # pad: close the current span
            if pos > span_start:
                spans.append((span_start, pos))
            span_start = start
        cur.append((tl, start))
        pos = start + w
    close_group()
    return groups


def build_nc(s=S, per_core=PER_CORE):
    """Build the SPMD single-core program (identical on all cores)."""
    nqb = s // QB

    nc = bacc.Bacc("TRN2", target_bir_lowering=False, debug=False)

    rqT = nc.declare_dram_parameter("rqT", [per_core, 128, s], dt.float16, isOutput=False)
    rkT = nc.declare_dram_parameter("rkT", [per_core, 128, s], dt.float16, isOutput=False)
    vT = nc.declare_dram_parameter("vT", [per_core, 128, s], dt.float16, isOutput=False)
    maskD = nc.declare_dram_parameter("maskD", [128, 128], dt.float16, isOutput=False)
    maskW12 = nc.declare_dram_parameter("maskW12", [128, 128], dt.float16, isOutput=False)
    maskW13 = nc.declare_dram_parameter("maskW13", [128, W13_W], dt.float16, isOutput=False)
    maskP = nc.declare_dram_parameter("maskP", [128, 128 + W13_W], dt.float16, isOutput=False)
    ones = nc.declare_dram_parameter("ones", [128, 128], dt.float16, isOutput=False)
    outT = nc.declare_dram_parameter("outT", [per_core, 128, s], dt.float16, isOutput=True)

    with tile.TileContext(nc) as tc:
        with (
            tc.tile_pool(name="const", bufs=1) as cpool,
            tc.tile_pool(name="big", bufs=2) as bigpool,
            tc.tile_pool(name="probs", bufs=4) as ppool,
            tc.tile_pool(name="acc", bufs=2) as apool,
            tc.tile_pool(name="outsb", bufs=3) as opool,
            tc.tile_pool(name="ps_sc", bufs=2, space="PSUM") as ps_sc,
            tc.tile_pool(name="ps_out", bufs=2, space="PSUM") as ps_out,
            tc.tile_pool(name="ps_den", bufs=2, space="PSUM") as ps_den,
        ):
            maskD_sb = cpool.tile([128, 128], dt.float16, tag="maskD")
            maskW12_sb = cpool.tile([128, 128], dt.float16, tag="maskW12")
            maskW13_sb = cpool.tile([128, W13_W], dt.float16, tag="maskW13")
            maskP_sb = cpool.tile([128, 128 + W13_W], dt.float16, tag="maskP")
            ones_sb = cpool.tile([128, 128], dt.float16, tag="ones")
            nc.sync.dma_start(out=maskD_sb[:], in_=maskD[:])
            nc.sync.dma_start(out=maskW12_sb[:], in_=maskW12[:])
            nc.sync.dma_start(out=maskW13_sb[:], in_=maskW13[:])
            nc.sync.dma_start(out=maskP_sb[:], in_=maskP[:])
            nc.sync.dma_start(out=ones_sb[:], in_=ones[:])

            def load(u, chunks):
                """Load unit u's rq/rk/v; chunks = list of (lo, hi) col ranges."""
                rq = bigpool.tile([128, s], dt.float16, tag="rq")
                rk = bigpool.tile([128, s], dt.float16, tag="rk")
                v = bigpool.tile([128, s], dt.float16, tag="v")
                for lo, hi in chunks:
                    sl = slice(lo, hi)
                    nc.sync.dma_start(out=rk[:, sl], in_=rkT[u][:, sl])
                    nc.sync.dma_start(out=rq[:, sl], in_=rqT[u][:, sl])
                    nc.sync.dma_start(out=v[:, sl], in_=vT[u][:, sl])
                return rq, rk, v

            def attention_qb(u, rq, rk, v, qb):
                tiles = qb_tiles(qb)
                # order: sink first (full 512, PV start covers all cols),
                # then remaining by descending eff width
                rest = sorted(tiles[1:], key=lambda t: -t["eff_w"])
                tiles = [tiles[0]] + rest
                assert tiles[0]["kj"] == 0 and tiles[0]["eff_w"] == 512
                groups = plan_groups(tiles)
                n_tiles = len(tiles)

                outT_ps = ps_out.tile([128, QB], dt.float32, tag="outT")
                S_sb = apool.tile([128, QB], dt.float16, tag="S")

                csl_base = qb * QB
                ti = 0
                for espans, gtiles in groups:
                    sc = ps_sc.tile([128, GROUP_W], dt.float32, tag="sc")
                    for tl, off in gtiles:
                        ksl = slice(tl["kj"] * 128, (tl["kj"] + 1) * 128)
                        csl = slice(csl_base + tl["t0"] * 128,
                                    csl_base + tl["t0"] * 128 + tl["eff_w"])
                        nc.tensor.matmul(
                            sc[:, off:off + tl["eff_w"]], rk[:, ksl], rq[:, csl],
                            start=True, stop=True,
                        )
                    probs = ppool.tile([128, GROUP_W], dt.float16, tag="probs")
                    for lo, hi in espans:
                        nc.scalar.activation(
                            probs[:, lo:hi], sc[:, lo:hi],
                            mybir.ActivationFunctionType.Exp, scale=SCALE,
                        )
                    # masks (on probs subranges), then accum + PV per tile
                    for tl, off in gtiles:
                        # collect masked chunk spans within this tile
                        mspans = []
                        for t, kind in tl["kinds"]:
                            if kind == "full":
                                continue
                            o = off + 128 * (t - tl["t0"])
                            mspans.append((o, kind))
                        j = 0
                        while j < len(mspans):
                            o, kind = mspans[j]
                            if (
                                kind == "w12"
                                and j + 1 < len(mspans)
                                and mspans[j + 1][1] == "w13"
                                and mspans[j + 1][0] == o + 128
                            ):
                                w = 128 + W13_W
                                nc.vector.tensor_mul(
                                    probs[:, o:o + w], probs[:, o:o + w], maskP_sb[:]
                                )
                                j += 2
                                continue
                            m, w = {
                                "diag": (maskD_sb, 128),
                                "w12": (maskW12_sb, 128),
                                "w13": (maskW13_sb, W13_W),
                            }[kind]
                            nc.vector.tensor_mul(
                                probs[:, o:o + w], probs[:, o:o + w], m[:, 0:w]
                            )
                            j += 1

                    for tl, off in gtiles:
                        w = tl["eff_w"]
                        psl = slice(off, off + w)
                        S_sl = slice(tl["t0"] * 128, tl["t0"] * 128 + w)
                        ksl = slice(tl["kj"] * 128, (tl["kj"] + 1) * 128)
                        first = ti == 0
                        last = ti == n_tiles - 1
                        # accumulate probs into S (denominator pre-sum)
                        if first:
                            nc.vector.tensor_copy(S_sb[:, S_sl], probs[:, psl])
                        elif w <= GSIMD_ACCUM_MAX_W:
                            nc.gpsimd.tensor_add(
                                S_sb[:, S_sl], S_sb[:, S_sl], probs[:, psl]
                            )
                        else:
                            nc.vector.tensor_add(
                                S_sb[:, S_sl], S_sb[:, S_sl], probs[:, psl]
                            )
                        # PV
                        osl = slice(tl["t0"] * 128, tl["t0"] * 128 + w)
                        nc.tensor.matmul(
                            outT_ps[:, osl], v[:, ksl], probs[:, psl],
                            start=first, stop=last,
                        )
                        ti += 1

                # tail: den = ones.T @ S (replicated over partitions), recip,
                # normalize, store
                den_ps = ps_den.tile([128, QB], dt.float32, tag="den")
                nc.tensor.matmul(den_ps[:], ones_sb[:], S_sb[:], start=True, stop=True)
                rden = opool.tile([128, QB], dt.float32, tag="rden")
                nc.vector.reciprocal_approx_fast(rden[:], den_ps[:])
                outN = opool.tile([128, QB], dt.float16, tag="outN")
                nc.vector.tensor_mul(outN[:], outT_ps[:], rden[:])
                nc.sync.dma_start(
                    out=outT[u][:, qb * QB:(qb + 1) * QB], in_=outN[:]
                )

            cur = load(0, [(0, 1024), (1024, 3072)])
            for u in range(per_core):
                nxt = None
                for qb in range(nqb):
                    attention_qb(u, cur[0], cur[1], cur[2], qb)
                    if qb == 0 and u + 1 < per_core:
                        nxt = load(u + 1, [(0, 3072)])
                cur = nxt

    nc.compile()
    return nc


def host_prep(q, k, v, cos, sin, s=S):
    """Rotary + per-core layouts on host. Returns (in_maps, units)."""
    b, _, h, d = q.shape

    # interleaved rotary, fp32 on host
    cos_t = cos.astype(np.float32)
    sin_t = sin.astype(np.float32)

    def rot(x):
        x1 = x[..., 0::2]
        x2 = x[..., 1::2]
        c = cos_t[None, :, None, :]
        sn = sin_t[None, :, None, :]
        o = np.empty_like(x)
        o[..., 0::2] = x1 * c - x2 * sn
        o[..., 1::2] = x2 * c + x1 * sn
        return o

    rq = rot(q.astype(np.float32)).astype(np.float16)
    rk = rot(k.astype(np.float32)).astype(np.float16)
    v16 = v.astype(np.float16)

    p = np.arange(128)[:, None]  # key within tile (partition)
    c = np.arange(128)[None, :]  # query within chunk (column)
    maskD = (c >= p).astype(np.float16)
    maskW12 = ((c - p) < T_W12).astype(np.float16)
    maskW13 = ((c[:, :W13_W] - p) < T_W13).astype(np.float16)
    maskP = np.concatenate([maskW12, maskW13], axis=1)
    ones = np.ones((128, 128), dtype=np.float16)

    units = [(bi, hi) for bi in range(b) for hi in range(h)]
    per = len(units) // N_CORES
    in_maps = []
    for core in range(N_CORES):
        us = units[core * per:(core + 1) * per]
        rqT = np.ascontiguousarray(np.stack([rq[bi, :, hi, :].T for bi, hi in us]))
        rkT = np.ascontiguousarray(np.stack([rk[bi, :, hi, :].T for bi, hi in us]))
        # v in [128 (s%128), NKT*128 (tile, d)] layout
        vT = np.ascontiguousarray(
            np.stack([
                v16[bi, :, hi, :].reshape(NKT, 128, 128).transpose(1, 0, 2)
                .reshape(128, s)
                for bi, hi in us
            ])
        )
        in_maps.append({
            "rqT": rqT, "rkT": rkT, "vT": vT,
            "maskD": maskD, "maskW12": maskW12, "maskW13": maskW13,
            "maskP": maskP, "ones": ones,
        })
    return in_maps, units


_NC_CACHE = {}


def kernel(q, k, v, cos, sin):
    from concourse.bass_utils import run_bass_kernel_spmd

    q = np.asarray(q, dtype=np.float32)
    k = np.asarray(k, dtype=np.float32)
    v = np.asarray(v, dtype=np.float32)
    cos = np.asarray(cos, dtype=np.float32)
    sin = np.asarray(sin, dtype=np.float32)

    if "nc" not in _NC_CACHE:
        _NC_CACHE["nc"] = build_nc()
    nc = _NC_CACHE["nc"]

    in_maps, units = host_prep(q, k, v, cos, sin)
    res = run_bass_kernel_spmd(nc, in_maps, core_ids=list(range(N_CORES)))

    b, s, h, d = q.shape
    full = np.empty((b, s, h, d), dtype=np.float32)
    per = len(units) // N_CORES
    for core in range(N_CORES):
        o = res.results[core]["outT"]  # [per, 128, s] fp16, transposed
        for i, (bi, hi) in enumerate(units[core * per:(core + 1) * per]):
            full[bi, :, hi, :] = o[i].T.astype(np.float32)
    return full
